# revision 1
# baseline (speedup 1.0000x reference)
"""Trainium2 Bass kernel for nn_CSA_84387517432661 (dense transformer
causal self-attention block: QKV proj + RMSNorm + RoPE + GQA causal SDPA +
output projection).

Sharding: tensor-parallel over heads across 8 NeuronCores. Core c owns
q-heads {2c, 2c+1} and kv-head c//2, computes its heads' attention output
for the full (B, T), AllGathers the per-head attention outputs (fp16), then
computes the output projection for its slice of 256 output columns.

Numerics: projections run on the PE in fp32r (full-rate fp32, ~1e-4),
attention and output projection in fp16 inputs with fp32 PSUM accumulation.
Softmax is computed without max-subtraction (scores are O(10), exp fits
fp16/fp32 comfortably) and normalization is folded in after attn@v.
"""

import sys

if "/opt/trn_rl_repo" not in sys.path:
    sys.path.insert(0, "/opt/trn_rl_repo")

import math

import numpy as np

NH = 16
NKV = 4
B = 2
D = 2048
HD = 128
N_CORES = 8
ROPE_BASE = 10000.0
ROPE_TSL = 1024
EPS = 1.1920928955078125e-07

_PROGRAM_CACHE = {}


def _rope_tables(T):
    rd = HD
    if T > ROPE_TSL:
        base = ROPE_BASE * (T / ROPE_TSL) ** (rd / (rd - 2))
    else:
        base = ROPE_BASE
    inv_freq = 1.0 / base ** (np.arange(0, rd, 2, dtype=np.float32) / rd)
    t = np.arange(T, dtype=np.float32)
    fr = np.outer(t, inv_freq)  # [T, 64]
    return np.cos(fr), np.sin(fr)


def _build_program(T, phases="ABCGE", reps=1):
    import concourse.mybir as mybir
    import concourse.tile as tile
    from concourse import bacc

    f32 = mybir.dt.float32
    f16 = mybir.dt.float16
    f32r = mybir.dt.float32r
    AF = mybir.ActivationFunctionType
    MUL = mybir.AluOpType.mult
    ADD = mybir.AluOpType.add

    BT = B * T
    NJ = T // 128        # 128-wide s-chunks per batch
    NQB = T // 512       # 512-wide q superblocks per batch
    NTS = BT // 1024     # 1024-wide col superblocks for the projections
    NDC = D // 128       # 128-row d chunks
    BT2 = BT // 2        # col half for out-proj
    NTB2 = BT2 // 512    # 512-wide t blocks per col half

    nc = bacc.Bacc("TRN2", target_bir_lowering=False, debug=False,
                   num_devices=N_CORES)

    xt_d = nc.declare_dram_parameter("xt", [D, BT], f32, isOutput=False)
    wqkv_d = nc.declare_dram_parameter("wqkv", [D, 512], f32, isOutput=False)
    wp_d = nc.declare_dram_parameter("wp", [D, 256], f16, isOutput=False)
    cost_d = nc.declare_dram_parameter("cost", [128, BT], f16, isOutput=False)
    sint_d = nc.declare_dram_parameter("sint", [128, BT], f16, isOutput=False)
    gain_d = nc.declare_dram_parameter("gain", [1, 2], f32, isOutput=False)
    mask_d = nc.declare_dram_parameter("masks", [128, 2048], f16, isOutput=False)
    id_d = nc.declare_dram_parameter("ident", [128, 128], f16, isOutput=False)
    out_d = nc.declare_dram_parameter("out", [256, BT], f32, isOutput=True)

    NCH = max(1, BT // 1024)   # AllGather chunks (overlap with attention)
    CW = BT // NCH
    y_contrib = [nc.dram_tensor(f"y_contrib{ch}", [256, CW], f16)
                 for ch in range(NCH)]
    y_all = [nc.dram_tensor(f"y_all{ch}", [2048, CW], f16, addr_space="Shared")
             for ch in range(NCH)]

    with tile.TileContext(nc) as tc:
        with (
            tc.tile_pool(name="consts", bufs=1) as cstp,
            tc.tile_pool(name="wts", bufs=4) as wtsp,
            tc.tile_pool(name="xs", bufs=3) as xsp,
            tc.tile_pool(name="big16", bufs=1) as bigp,
            tc.tile_pool(name="bwork", bufs=2) as bwp,
            tc.tile_pool(name="cstream", bufs=2) as csp,
            tc.tile_pool(name="a16", bufs=1) as a16p,
            tc.tile_pool(name="ep", bufs=6) as epp,
            tc.tile_pool(name="cnorm", bufs=2) as cnp,
            tc.tile_pool(name="wpp", bufs=16) as wpp,
            tc.tile_pool(name="yr", bufs=2) as yrp,
            tc.tile_pool(name="oev", bufs=4) as oevp,
            tc.tile_pool(name="pp", bufs=8, space="PSUM") as pp,
        ):
            # ---- constants ----
            mask_t = cstp.tile([128, 2048], f16, tag="mask")
            nc.sync.dma_start(out=mask_t[:], in_=mask_d.ap())
            id_t = cstp.tile([128, 128], f16, tag="id")
            nc.sync.dma_start(out=id_t[:], in_=id_d.ap())
            g_t = cstp.tile([1, 2], f32, tag="g")
            nc.sync.dma_start(out=g_t[:], in_=gain_d.ap())
            ones16 = cstp.tile([128, 1], f16, tag="ones")
            nc.vector.memset(ones16[:], 1.0)
            eps_t = cstp.tile([1, 1], f32, tag="eps")
            nc.vector.memset(eps_t[:], EPS)

            for _rep in range(reps):
                if _rep > 0:
                    tc.strict_bb_all_engine_barrier()
                # ---- phase A: QKV projections (fp32r), outputs transposed ----
                names = ("q0", "q1", "k", "v")
                big = {}
                for name in names:
                    big[name] = bigp.tile([128, BT], f16, tag=name, name="big_" + name)

                for ts in range(NTS):
                    ps = [[pp.tile([128, 512], f32, tag="ps", name=f"pa_{cg}_{tb}")
                           for tb in range(2)] for cg in range(4)]
                    for j in range(NDC):
                        wt = wtsp.tile([128, 512], f32r, tag="w")
                        nc.sync.dma_start(
                            out=wt[:],
                            in_=wqkv_d.ap()[j * 128:(j + 1) * 128, :].bitcast(f32r))
                        xt_t = xsp.tile([128, 1024], f32r, tag="x")
                        nc.sync.dma_start(
                            out=xt_t[:],
                            in_=xt_d.ap()[j * 128:(j + 1) * 128,
                                          ts * 1024:(ts + 1) * 1024].bitcast(f32r))
                        for cg in range(4):
                            for tb in range(2):
                                nc.tensor.matmul(
                                    ps[cg][tb][:],
                                    wt[:, cg * 128:(cg + 1) * 128],
                                    xt_t[:, tb * 512:(tb + 1) * 512],
                                    start=(j == 0), stop=(j == NDC - 1))
                    for cg, name in enumerate(names):
                        for tb in range(2):
                            c0 = ts * 1024 + tb * 512
                            nc.scalar.copy(big[name][:, c0:c0 + 512], ps[cg][tb][:])

                # ---- phase B: RMSNorm + RoPE (+gain), cast fp16; v transpose ----
                a16 = {}
                for name in ("q0", "q1", "k"):
                    a16[name] = a16p.tile([128, BT], f16, tag=name,
                                          name="a16_" + name)
                for idx, name in enumerate(("q0", "q1", "k")):
                    srcb = big[name]
                    o16 = a16[name]
                    for cb in range(BT // 1024):
                        c0 = cb * 1024
                        sblk = srcb[:, c0:c0 + 1024]
                        sq16 = bwp.tile([128, 1024], f16, tag="sq")
                        nc.scalar.activation(sq16[:], sblk, AF.Square)
                        rs = bwp.tile([1, 1024], f32, tag="rs")
                        for sub in range(2):
                            ssq = pp.tile([1, 512], f32, tag="ps")
                            nc.tensor.matmul(ssq[:], ones16[:],
                                             sq16[:, sub * 512:(sub + 1) * 512],
                                             start=True, stop=True)
                            nc.scalar.activation(rs[0:1, sub * 512:(sub + 1) * 512],
                                                 ssq[:], AF.Sqrt, scale=1.0 / HD,
                                                 bias=eps_t[0:1, 0:1])
                        nc.vector.reciprocal(rs[:], rs[:])
                        if name != "k":
                            nc.vector.tensor_scalar_mul(rs[:], rs[:],
                                                        g_t[0:1, idx:idx + 1])
                        rs16 = bwp.tile([1, 1024], f16, tag="rs16")
                        nc.vector.tensor_copy(rs16[:], rs[:])
                        rsb = bwp.tile([128, 1024], f16, tag="rsb")
                        nc.gpsimd.partition_broadcast(rsb[:], rs16[:])
                        qsw = bwp.tile([128, 1024], f16, tag="qsw")
                        nc.sync.dma_start(out=qsw[0:64, :], in_=sblk[64:128, :])
                        nc.sync.dma_start(out=qsw[64:128, :], in_=sblk[0:64, :])
                        cos_b = csp.tile([128, 1024], f16, tag="cosb")
                        nc.sync.dma_start(out=cos_b[:],
                                          in_=cost_d.ap()[:, c0:c0 + 1024])
                        sin_b = csp.tile([128, 1024], f16, tag="sinb")
                        nc.sync.dma_start(out=sin_b[:],
                                          in_=sint_d.ap()[:, c0:c0 + 1024])
                        m1 = bwp.tile([128, 1024], f16, tag="m1")
                        nc.vector.tensor_tensor(m1[:], sblk, cos_b[:], op=MUL)
                        m2 = bwp.tile([128, 1024], f16, tag="m2")
                        nc.vector.tensor_tensor(m2[:], qsw[:], sin_b[:], op=MUL)
                        nc.vector.tensor_tensor(m1[:], m1[:], m2[:], op=ADD)
                        nc.vector.tensor_tensor(o16[:, c0:c0 + 1024], m1[:],
                                                rsb[:], op=MUL)

                v16 = a16p.tile([128, BT], f16, tag="v")
                for b in range(B):
                    for j in range(NJ):
                        tp_ = pp.tile([128, 128], f16, tag="ps")
                        nc.tensor.transpose(
                            tp_[:], big["v"][:, b * T + j * 128: b * T + (j + 1) * 128],
                            id_t[:])
                        c0 = (b * NJ + j) * 128
                        nc.vector.tensor_copy(v16[:, c0:c0 + 128], tp_[:])

                # ---- phase C: causal attention per (batch, q superblock) ----
                inv_sqrt_hd = 1.0 / math.sqrt(HD)
                k16 = a16["k"]
                for b in range(B if "C" in phases else 0):
                    for Q in range(NQB):
                        njq = 4 * Q + 4
                        yts = [pp.tile([128, 512], f32, tag="ps", name=f"yt{h}") for h in range(2)]
                        zs = [pp.tile([1, 512], f32, tag="ps", name=f"z{h}") for h in range(2)]
                        for j in range(njq):
                            es = []
                            for h in range(2):
                                sc = pp.tile([128, 512], f32, tag="ps")
                                nc.tensor.matmul(
                                    sc[:],
                                    k16[:, b * T + j * 128: b * T + (j + 1) * 128],
                                    a16["q0" if h == 0 else "q1"][
                                        :, b * T + Q * 512: b * T + (Q + 1) * 512],
                                    start=True, stop=True)
                                e = epp.tile([128, 512], f16, tag="e")
                                nc.scalar.activation(e[:], sc[:], AF.Exp,
                                                     scale=inv_sqrt_hd)
                                if j >= 4 * Q:
                                    r = j - 4 * Q
                                    nc.vector.tensor_tensor(
                                        e[:], e[:], mask_t[:, r * 512:(r + 1) * 512],
                                        op=MUL)
                                es.append(e)
                            vslice = v16[:, (b * NJ + j) * 128:(b * NJ + j + 1) * 128]
                            for h in range(2):
                                nc.tensor.matmul(yts[h][:], vslice, es[h][:],
                                                 start=(j == 0), stop=(j == njq - 1))
                            for h in range(2):
                                nc.tensor.matmul(zs[h][:], ones16[:], es[h][:],
                                                 start=(j == 0), stop=(j == njq - 1))
                        for h in range(2):
                            rz = cnp.tile([1, 512], f32, tag="rz")
                            nc.vector.reciprocal(rz[:], zs[h][:])
                            rzb = cnp.tile([128, 512], f32, tag="rzb")
                            nc.gpsimd.partition_broadcast(rzb[:], rz[:])
                            y16 = cnp.tile([128, 512], f16, tag="y16")
                            nc.vector.tensor_tensor(y16[:], yts[h][:], rzb[:], op=MUL)
                            cg0 = b * T + Q * 512
                            ch, cc0 = cg0 // CW, cg0 % CW
                            nc.sync.dma_start(
                                out=y_contrib[ch].ap()[h * 128:(h + 1) * 128,
                                                       cc0:cc0 + 512],
                                in_=y16[:])
                        if "G" in phases and (b * T + (Q + 1) * 512) % CW == 0:
                            chd = (b * T + Q * 512) // CW
                            nc.gpsimd.collective_compute(
                                "AllGather", mybir.AluOpType.bypass,
                                replica_groups=[list(range(N_CORES))],
                                ins=[y_contrib[chd].ap()],
                                outs=[y_all[chd].ap()])

                # ---- phase E: output projection for this core's 256 columns ----
                if "E" not in phases:
                    dummy = oevp.tile([128, 512], f32, tag="oe")
                    nc.vector.memset(dummy[:], 0.0)
                    for eh in range(2):
                        for cb2 in range(BT // 512):
                            nc.sync.dma_start(
                                out=out_d.ap()[eh * 128:(eh + 1) * 128,
                                               cb2 * 512:(cb2 + 1) * 512],
                                in_=dummy[:])
                wp_tiles = []
                if "E" not in phases:
                    wp_range = 0
                else:
                    wp_range = NDC
                for dvc in range(wp_range):
                    wpt = wpp.tile([128, 256], f16, tag="wp")
                    nc.sync.dma_start(out=wpt[:],
                                      in_=wp_d.ap()[dvc * 128:(dvc + 1) * 128, :])
                    wp_tiles.append(wpt)

                NTBC = CW // 512
                for th in range(NCH if "E" in phases else 0):
                    pso = [[pp.tile([128, 512], f32, tag="ps", name=f"po_{eh}_{tb}")
                            for tb in range(NTBC)] for eh in range(2)]
                    for dvc in range(NDC):
                        yr_t = yrp.tile([128, CW], f16, tag="yr")
                        nc.sync.dma_start(
                            out=yr_t[:],
                            in_=y_all[th].ap()[dvc * 128:(dvc + 1) * 128, :])
                        for eh in range(2):
                            for tb in range(NTBC):
                                nc.tensor.matmul(
                                    pso[eh][tb][:],
                                    wp_tiles[dvc][:, eh * 128:(eh + 1) * 128],
                                    yr_t[:, tb * 512:(tb + 1) * 512],
                                    start=(dvc == 0), stop=(dvc == NDC - 1))
                    for eh in range(2):
                        for tb in range(NTBC):
                            ot = oevp.tile([128, 512], f32, tag="oe")
                            nc.scalar.copy(ot[:], pso[eh][tb][:])
                            c0 = th * CW + tb * 512
                            nc.sync.dma_start(
                                out=out_d.ap()[eh * 128:(eh + 1) * 128, c0:c0 + 512],
                                in_=ot[:])

    nc.finalize()
    return nc


def _prepare_in_maps(x, Wq, Wk, Wv, Wp, q_gain):
    Bx, T, Dx = x.shape
    assert (Bx, Dx) == (B, D)
    BT = B * T

    x = np.asarray(x, dtype=np.float32)
    Wq = np.asarray(Wq, dtype=np.float32)
    Wk = np.asarray(Wk, dtype=np.float32)
    Wv = np.asarray(Wv, dtype=np.float32)
    Wp = np.asarray(Wp, dtype=np.float32)
    q_gain = np.asarray(q_gain, dtype=np.float32)

    xt_np = np.ascontiguousarray(x.reshape(BT, D).T)  # [D, BT]

    cos_, sin_ = _rope_tables(T)  # [T, 64]
    ct = np.tile(cos_.T, (1, B))  # [64, BT]
    st = np.tile(sin_.T, (1, B))
    cost_np = np.ascontiguousarray(np.vstack([ct, ct]).astype(np.float16))
    sint_np = np.ascontiguousarray(np.vstack([st, -st]).astype(np.float16))

    s_idx = np.arange(128)[:, None]
    q_idx = np.arange(512)[None, :]
    masks_np = np.concatenate(
        [(q_idx >= 128 * r + s_idx).astype(np.float16) for r in range(4)],
        axis=1)  # [128, 2048]
    masks_np = np.ascontiguousarray(masks_np)
    ident_np = np.eye(128, dtype=np.float16)

    in_maps = []
    for c in range(N_CORES):
        h0, h1 = 2 * c, 2 * c + 1
        kv = c // 2
        wqkv_np = np.ascontiguousarray(np.concatenate([
            Wq[h0 * HD:(h0 + 1) * HD],
            Wq[h1 * HD:(h1 + 1) * HD],
            Wk[kv * HD:(kv + 1) * HD],
            Wv[kv * HD:(kv + 1) * HD],
        ], axis=0).T)  # [D, 512]
        wp_np = np.ascontiguousarray(
            Wp[c * 256:(c + 1) * 256, :].T.astype(np.float16))  # [D, 256]
        gain_np = np.ascontiguousarray(q_gain[h0:h1 + 1].reshape(1, 2))
        in_maps.append({
            "xt": xt_np,
            "wqkv": wqkv_np,
            "wp": wp_np,
            "cost": cost_np,
            "sint": sint_np,
            "gain": gain_np,
            "masks": masks_np,
            "ident": ident_np,
        })
    return in_maps


def _assemble_output(results, T):
    BT = B * T
    full = np.concatenate([results[c]["out"] for c in range(N_CORES)],
                          axis=0)  # [2048, BT] = out transposed
    return np.ascontiguousarray(
        full.reshape(D, B, T).transpose(1, 2, 0)).astype(np.float32)


def run_on_hw(x, Wq, Wk, Wv, Wp, q_gain, trace=False):
    from concourse.bass_utils import run_bass_kernel_spmd

    T = x.shape[1]
    if T not in _PROGRAM_CACHE:
        _PROGRAM_CACHE[T] = _build_program(T)
    nc = _PROGRAM_CACHE[T]
    in_maps = _prepare_in_maps(x, Wq, Wk, Wv, Wp, q_gain)
    res = run_bass_kernel_spmd(nc, in_maps, list(range(N_CORES)), trace=trace)
    out = _assemble_output(res.results, T)
    return out, res


def kernel(x, Wq, Wk, Wv, Wp, q_gain):
    out, _ = run_on_hw(x, Wq, Wk, Wv, Wp, q_gain, trace=False)
    return out

